# revision 9
# baseline (speedup 1.0000x reference)
"""Causal self-attention with RoPE on 8 trn2 NeuronCores.

Sharding: core = (batch, head-half). Each of the 8 cores handles one batch
(b = core//2) and 6 of the 12 heads (hh = core%2). Each core computes a
partial output projection (its heads' contribution to y @ Wproj); the host
sums the two partials per batch.

Device kernel (identical SPMD program on every core), emission interleaved so
attention starts as soon as its head-pair inputs are ready:
  - v = x @ Wv in natural s-major layout (fp16), with an appended ones column
    per head whose matmul row yields the softmax denominator for free.
  - qT/kT = (x @ Wq/Wk)^T in d-major float32r with RoPE applied via
    stream_shuffle: the head dim is pre-permuted host-side so rotate-half
    partners sit in adjacent even/odd lanes (scores are invariant to that
    permutation).
  - flash-style causal attention per head-pair: S^T blocks (k-partition x
    q-free) via row-packed K=64 matmuls, one 3D-AP exp per (pair, chunk, kb)
    covering both heads, diagonal mask multiply on gpsimd, and
    yT[65 x q] = [v|1]^T @ E accumulated in PSUM (row 64 = denominator).
  - out_partial = (yT / denom)^T @ Wp rows.
All matmuls run in float32r (full-rate PE); E/v are fp16.
"""
import contextlib

import numpy as np

import concourse.bacc as bacc
import concourse.mybir as mybir
import concourse.tile as tile
from concourse import bass_utils

F32 = mybir.dt.float32
F32R = mybir.dt.float32r
F16 = mybir.dt.float16

B, S, C, H, D = 4, 2048, 768, 12, 64
HPC = H // 2          # heads per core = 6
HP = HPC // 2         # head pairs per core = 3
KC = C // 128         # contraction tiles over C = 6
NST = S // 128        # 128-row s tiles = 16
NSC = S // 512        # 512-wide s chunks = 4
ROPE_BASE = 10000.0

EVEN_ODD_MASK = [x for j in range(16) for x in (2 * j + 1, 2 * j)]


def build_program():
    nc = bacc.Bacc("TRN2", target_bir_lowering=False, debug=False)
    xT_d = nc.dram_tensor("xT", [C, S], F16, kind="ExternalInput").ap()
    wqk_d = nc.dram_tensor("wqk", [C, 768], F16, kind="ExternalInput").ap()
    wv_d = nc.dram_tensor("wv", [C, 384], F16, kind="ExternalInput").ap()
    wp_d = nc.dram_tensor("wp", [384, C], F32R, kind="ExternalInput").ap()
    cos_d = nc.dram_tensor("cosT", [128, S], F16, kind="ExternalInput").ap()
    sin_d = nc.dram_tensor("sinA", [128, S], F16, kind="ExternalInput").ap()
    mask_d = nc.dram_tensor("mask01", [128, 128], F16, kind="ExternalInput").ap()
    out_d = nc.dram_tensor("out", [S, C], F32, kind="ExternalOutput").ap()

    with tile.TileContext(nc) as tc, contextlib.ExitStack() as top:
        sb = top.enter_context(tc.tile_pool(name="sb", bufs=1))
        ps = top.enter_context(tc.tile_pool(name="ps", bufs=1, space="PSUM"))

        qkT = [sb.tile([128, S], F32R, name=f"qkT{i}", tag=f"qkT{i}") for i in range(6)]
        vones = [
            sb.tile([128, HPC * 65], F16, name=f"vones{i}", tag=f"vones{i}")
            for i in range(NST)
        ]
        yTn = [sb.tile([128, S], F32R, name=f"yTn{i}", tag=f"yTn{i}") for i in range(HP)]
        mask01 = sb.tile([128, 128], F16, name="mask01", tag="mask01")
        xT = [sb.tile([128, S], F16, name=f"xT{i}", tag="xT", bufs=KC) for i in range(KC)]
        wqk = [
            sb.tile([128, 768], F16, name=f"wqk{i}", tag="wqk", bufs=KC)
            for i in range(KC)
        ]
        wv = [
            sb.tile([128, 384], F16, name=f"wv{i}", tag="wv", bufs=KC)
            for i in range(KC)
        ]
        cosT = sb.tile([128, S], F16, name="cosT", tag="cosT")
        sinA = sb.tile([128, S], F16, name="sinA", tag="sinA")
        ones6 = sb.tile([128, HPC], F16, name="ones6", tag="ones6")

        for i in range(KC):
            nc.sync.dma_start(wqk[i][:], wqk_d[128 * i : 128 * (i + 1), :])
            nc.sync.dma_start(xT[i][:], xT_d[128 * i : 128 * (i + 1), :])
            nc.sync.dma_start(wv[i][:], wv_d[128 * i : 128 * (i + 1), :])
        nc.sync.dma_start(cosT[:], cos_d[:])
        nc.sync.dma_start(sinA[:], sin_d[:])
        nc.sync.dma_start(mask01[:], mask_d[:])
        nc.gpsimd.memset(ones6[:], 1.0)

        def qk_tile(m):
            """(x @ Wq/Wk)^T m-tile with RoPE, into qkT[m]."""
            for sc in range(NSC):
                sl = slice(512 * sc, 512 * (sc + 1))
                qkps_t = ps.tile([128, 512], F32, name="qkps", tag="pq", bufs=2)
                qkps = qkps_t[:, 0:512]
                for kc in range(KC):
                    nc.tensor.matmul(
                        qkps,
                        wqk[kc][:, 128 * m : 128 * (m + 1)],
                        xT[kc][:, sl],
                        start=(kc == 0),
                        stop=(kc == KC - 1),
                    )
                shuf = sb.tile([128, 512], F32, name="shuf", tag="shuf", bufs=3)
                nc.vector.stream_shuffle(shuf[:], qkps, EVEN_ODD_MASK)
                nc.vector.tensor_mul(qkT[m][:, sl], qkps, cosT[:, sl])
                nc.gpsimd.tensor_mul(shuf[:], shuf[:], sinA[:, sl])
                nc.vector.tensor_add(qkT[m][:, sl], qkT[m][:, sl], shuf[:])

        def v_tile(st):
            """v s-tile (bf16, with ones columns) into vones[st]."""
            vps_t = ps.tile([128, 512], F32, name="vps", tag="pq", bufs=2)
            vps = vps_t[:, 0:384]
            for kc in range(KC):
                nc.tensor.matmul(
                    vps,
                    xT[kc][:, 128 * st : 128 * (st + 1)],
                    wv[kc][:],
                    start=(kc == 0),
                    stop=(kc == KC - 1),
                )
            v3 = vones[st][:].rearrange("p (h w) -> p h w", w=65)
            nc.scalar.copy(v3[:, :, 0:64], vps.rearrange("p (h w) -> p h w", w=64))
            nc.scalar.copy(v3[:, :, 64:65], ones6[:].unsqueeze(2))

        def attn_chunk(hp, c):
            """Causal attention for q-chunk c of head pair hp -> yTn slice."""
            qTt, kTt = qkT[hp], qkT[HP + hp]
            yps = [
                ps.tile([128, 512], F32, name="yps", tag="yT", bufs=2) for _ in range(2)
            ]
            for kb in range(4 * c + 4):
                off = max(0, 128 * kb - 512 * c)
                qsl = slice(512 * c + off, 512 * (c + 1))
                ksl = slice(128 * kb, 128 * (kb + 1))
                sT = ps.tile([128, 1024], F32, name="sT", tag="sT", bufs=2)
                nc.tensor.matmul(
                    sT[:, off:512], kTt[0:64, ksl], qTt[0:64, qsl],
                    start=True, stop=True, tile_position=(0, 0),
                )
                nc.tensor.matmul(
                    sT[:, 512 + off : 1024], kTt[64:128, ksl], qTt[64:128, qsl],
                    start=True, stop=True, tile_position=(64, 0),
                )
                eT = sb.tile([128, 1024], F16, name="eT", tag="eT", bufs=4)
                in3 = sT[:].rearrange("p (b w) -> p b w", b=2)[:, :, off:512]
                out3 = eT[:].rearrange("p (b w) -> p b w", b=2)[:, :, off:512]
                nc.scalar.activation(
                    out3, in3, mybir.ActivationFunctionType.Exp, scale=D**-0.5
                )
                if kb >= 4 * c:  # diagonal block: causal mask multiply
                    for h in range(2):
                        dsl = slice(512 * h + off, 512 * h + off + 128)
                        nc.vector.tensor_mul(eT[:, dsl], eT[:, dsl], mask01[:])
                for h in range(2):
                    nc.tensor.matmul(
                        yps[h][0:65, off:512],
                        vones[kb][:, 65 * (2 * hp + h) : 65 * (2 * hp + h) + 65],
                        eT[:, 512 * h + off : 512 * (h + 1)],
                        start=(kb == 0),
                        stop=(kb == 4 * c + 3),
                    )
            for h in range(2):
                recip = sb.tile([1, 512], F32, name="recip", tag="recip", bufs=1)
                nc.vector.reciprocal(recip[:], yps[h][64:65, 0:512])
                bc = sb.tile([64, 512], F32, name="bc", tag="bc", bufs=1)
                nc.gpsimd.partition_broadcast(bc[:], recip[:], channels=64)
                nc.vector.tensor_mul(
                    yTn[hp][64 * h : 64 * (h + 1), 512 * c : 512 * (c + 1)],
                    yps[h][0:64, 0:512],
                    bc[:],
                )

        # interleaved emission: attention preferred (high_priority) as soon as
        # its deps allow; qk/v projection tiles gap-fill the PE
        def attn_hi(hp, c):
            with tc.high_priority(offset=150):
                attn_chunk(hp, c)

        qk_tile(0)
        qk_tile(3)
        for st in range(0, 4):
            v_tile(st)
        attn_hi(0, 0)
        qk_tile(1)
        for st in range(4, 8):
            v_tile(st)
        attn_hi(0, 1)
        qk_tile(4)
        for st in range(8, 12):
            v_tile(st)
        attn_hi(0, 2)
        qk_tile(2)
        for st in range(12, 16):
            v_tile(st)
        attn_hi(0, 3)
        qk_tile(5)
        for c in range(NSC):
            attn_chunk(1, c)
        for c in range(NSC):
            attn_chunk(2, c)

        wp = [
            sb.tile([128, 768], F32R, name=f"wp{i}", tag="wp", bufs=HP)
            for i in range(HP)
        ]
        for i in range(HP):
            nc.sync.dma_start(wp[i][:], wp_d[128 * i : 128 * (i + 1), :])
        for st in range(NST):
            osb = sb.tile([128, 768], F32, name="osb", tag="xT", bufs=KC)
            for half in range(2):
                ops_t = ps.tile([128, 512], F32, name="ops", tag="pq", bufs=2)
                ops_ = ops_t[:, 0:384]
                for t in range(HP):
                    nc.tensor.matmul(
                        ops_,
                        yTn[t][:, 128 * st : 128 * (st + 1)],
                        wp[t][:, 384 * half : 384 * (half + 1)],
                        start=(t == 0),
                        stop=(t == HP - 1),
                    )
                nc.scalar.copy(osb[:, 384 * half : 384 * (half + 1)], ops_)
            nc.sync.dma_start(out_d[128 * st : 128 * (st + 1), :], osb[:])

    nc.compile()
    return nc


def _rope_tables():
    """cosT/sinA in the even/odd-interleaved d order, tiled to 128 partitions."""
    j = np.arange(32, dtype=np.float64)
    theta = ROPE_BASE ** (-2.0 * j / D)
    pos = np.arange(S, dtype=np.float64)
    freqs = np.outer(theta, pos)  # (32, S)
    cos = np.cos(freqs)
    sin = np.sin(freqs)
    cosT = np.empty((64, S), np.float32)
    sinA = np.empty((64, S), np.float32)
    cosT[0::2] = cos
    cosT[1::2] = cos
    sinA[0::2] = -sin
    sinA[1::2] = sin
    return np.tile(cosT, (2, 1)).copy(), np.tile(sinA, (2, 1)).copy()


def _head_perm():
    """Even/odd interleave of RoPE partner dims, per head (384 cols)."""
    perm = np.empty(384, np.int64)
    for h in range(HPC):
        for j in range(32):
            perm[64 * h + 2 * j] = 64 * h + j
            perm[64 * h + 2 * j + 1] = 64 * h + j + 32
    return perm


def make_in_maps(x, Wqkv, Wproj):
    x = np.asarray(x, np.float32)
    Wqkv = np.asarray(Wqkv, np.float32)
    Wproj = np.asarray(Wproj, np.float32)
    wq, wk, wv = Wqkv[:, 0:C], Wqkv[:, C : 2 * C], Wqkv[:, 2 * C : 3 * C]
    cosT, sinA = _rope_tables()
    perm = _head_perm()
    mask01 = (np.arange(128)[None, :] >= np.arange(128)[:, None]).astype(
        np.float16
    )
    in_maps = []
    for core in range(8):
        b, hh = core // 2, core % 2
        cols = slice(384 * hh, 384 * (hh + 1))
        wq_c = wq[:, cols][:, perm]
        wk_c = wk[:, cols][:, perm]
        in_maps.append(
            {
                "xT": np.ascontiguousarray(x[b].T.astype(np.float16)),
                "wqk": np.ascontiguousarray(np.concatenate([wq_c, wk_c], axis=1).astype(np.float16)),
                "wv": np.ascontiguousarray(wv[:, cols].astype(np.float16)),
                "wp": np.ascontiguousarray(Wproj[384 * hh : 384 * (hh + 1), :]),
                "cosT": cosT.astype(np.float16),
                "sinA": sinA.astype(np.float16),
                "mask01": mask01,
            }
        )
    return in_maps


_NC_CACHE = None


def _get_program():
    global _NC_CACHE
    if _NC_CACHE is None:
        _NC_CACHE = build_program()
    return _NC_CACHE


def kernel(x, Wqkv, Wproj):
    nc = _get_program()
    in_maps = make_in_maps(x, Wqkv, Wproj)
    res = bass_utils.run_bass_kernel_spmd(nc, in_maps, core_ids=list(range(8)))
    out = np.empty((B, S, C), np.float32)
    for b in range(B):
        out[b] = res.results[2 * b]["out"] + res.results[2 * b + 1]["out"]
    return out


# revision 19
# speedup vs baseline: 456.6932x; 456.6932x over previous
"""Causal self-attention with RoPE on 8 trn2 NeuronCores.

Sharding: core = (batch, head-half). Each of the 8 cores handles one batch
(b = core//2) and 6 of the 12 heads (hh = core%2). Each core computes a
partial output projection (its heads' contribution to y @ Wproj); the host
sums the two partials per batch.

Device kernel (identical SPMD program on every core), emission interleaved so
attention starts as soon as its head-pair inputs are ready:
  - v = x @ Wv in natural s-major layout (fp16), with an appended ones column
    per head whose matmul row yields the softmax denominator for free.
  - qT/kT = (x @ Wq/Wk)^T in d-major float32r with RoPE applied via
    stream_shuffle: the head dim is pre-permuted host-side so rotate-half
    partners sit in adjacent even/odd lanes (scores are invariant to that
    permutation).
  - flash-style causal attention per head-pair: S^T blocks (k-partition x
    q-free) via row-packed K=64 matmuls, one 3D-AP exp per (pair, chunk, kb)
    covering both heads, diagonal mask multiply on gpsimd, and
    yT[65 x q] = [v|1]^T @ E accumulated in PSUM (row 64 = denominator).
  - out_partial = (yT / denom)^T @ Wp rows.
All matmuls run in float32r (full-rate PE); E/v are fp16.
"""
import contextlib

import numpy as np

import concourse.bacc as bacc
import concourse.mybir as mybir
import concourse.tile as tile
from concourse import bass_utils

F32 = mybir.dt.float32
F32R = mybir.dt.float32r
F16 = mybir.dt.float16

B, S, C, H, D = 4, 2048, 768, 12, 64
HPC = H // 2          # heads per core = 6
HP = HPC // 2         # head pairs per core = 3
KC = C // 128         # contraction tiles over C = 6
NST = S // 128        # 128-row s tiles = 16
NSC = S // 512        # 512-wide s chunks = 4
ROPE_BASE = 10000.0

EVEN_ODD_MASK = [x for j in range(16) for x in (2 * j + 1, 2 * j)]


def build_program():
    nc = bacc.Bacc("TRN2", target_bir_lowering=False, debug=False)
    xT_d = nc.dram_tensor("xT", [C, S], F16, kind="ExternalInput").ap()
    wqk_d = nc.dram_tensor("wqk", [C, 768], F16, kind="ExternalInput").ap()
    wv_d = nc.dram_tensor("wv", [C, 384], F16, kind="ExternalInput").ap()
    wp_d = nc.dram_tensor("wp", [384, C], F32R, kind="ExternalInput").ap()
    cos_d = nc.dram_tensor("cosT", [128, S], F16, kind="ExternalInput").ap()
    sin_d = nc.dram_tensor("sinA", [128, S], F16, kind="ExternalInput").ap()
    mask_d = nc.dram_tensor("mask01", [128, 128], F16, kind="ExternalInput").ap()
    out_d = nc.dram_tensor("out", [S, C], F32, kind="ExternalOutput").ap()

    with tile.TileContext(nc) as tc, contextlib.ExitStack() as top:
        sb = top.enter_context(tc.tile_pool(name="sb", bufs=1))
        ps = top.enter_context(tc.tile_pool(name="ps", bufs=1, space="PSUM"))

        qkT = [sb.tile([128, S], F32R, name=f"qkT{i}", tag=f"qkT{i}") for i in range(6)]
        vones = [
            sb.tile([128, HPC * 65], F16, name=f"vones{i}", tag=f"vones{i}")
            for i in range(NST)
        ]
        yTn = [sb.tile([128, S], F32R, name=f"yTn{i}", tag=f"yTn{i}") for i in range(HP)]
        mask01 = sb.tile([128, 128], F16, name="mask01", tag="mask01")
        xT = [sb.tile([128, S], F16, name=f"xT{i}", tag="xT", bufs=KC) for i in range(KC)]
        wqk = [
            sb.tile([128, 768], F16, name=f"wqk{i}", tag="wqk", bufs=KC)
            for i in range(KC)
        ]
        wv = [
            sb.tile([128, 384], F16, name=f"wv{i}", tag="wv", bufs=KC)
            for i in range(KC)
        ]
        cosT = sb.tile([128, S], F16, name="cosT", tag="cosT")
        sinA = sb.tile([128, S], F16, name="sinA", tag="sinA")
        ones6 = sb.tile([128, HPC], F16, name="ones6", tag="ones6")

        # load order tuned so the first attention chunk's deps land first:
        # weights, then xT/cos/sin chunk-by-chunk (sc=0 first)
        nc.sync.dma_start(mask01[:], mask_d[:])
        for i in range(KC):
            nc.sync.dma_start(wqk[i][:], wqk_d[128 * i : 128 * (i + 1), :])
        for sc in range(NSC):
            sl = slice(512 * sc, 512 * (sc + 1))
            for i in range(KC):
                nc.sync.dma_start(xT[i][:, sl], xT_d[128 * i : 128 * (i + 1), sl])
            nc.sync.dma_start(cosT[:, sl], cos_d[:, sl])
            nc.sync.dma_start(sinA[:, sl], sin_d[:, sl])
            if sc == 0:
                for i in range(KC):
                    nc.sync.dma_start(wv[i][:], wv_d[128 * i : 128 * (i + 1), :])
        nc.gpsimd.memset(ones6[:], 1.0)

        def qk_sc(m, sc):
            """(x @ Wq/Wk)^T m-tile, q-chunk sc, with RoPE, into qkT[m]."""
            sl = slice(512 * sc, 512 * (sc + 1))
            qkps_t = ps.tile([128, 512], F32, name="qkps", tag="pq", bufs=2)
            qkps = qkps_t[:, 0:512]
            for kc in range(KC):
                nc.tensor.matmul(
                    qkps,
                    wqk[kc][:, 128 * m : 128 * (m + 1)],
                    xT[kc][:, sl],
                    start=(kc == 0),
                    stop=(kc == KC - 1),
                )
            qksb = sb.tile([128, 512], F32, name="qksb", tag="qksb", bufs=3)
            nc.scalar.copy(qksb[:], qkps)
            shuf = sb.tile([128, 512], F32, name="shuf", tag="shuf", bufs=3)
            nc.vector.stream_shuffle(shuf[:], qksb[:], EVEN_ODD_MASK)
            nc.vector.tensor_mul(qkT[m][:, sl], qksb[:], cosT[:, sl])
            nc.gpsimd.tensor_mul(shuf[:], shuf[:], sinA[:, sl])
            nc.vector.tensor_add(qkT[m][:, sl], qkT[m][:, sl], shuf[:])

        def v_tile(st):
            """v s-tile (bf16, with ones columns) into vones[st]."""
            vps_t = ps.tile([128, 512], F32, name="vps", tag="pq", bufs=2)
            vps = vps_t[:, 0:384]
            for kc in range(KC):
                nc.tensor.matmul(
                    vps,
                    xT[kc][:, 128 * st : 128 * (st + 1)],
                    wv[kc][:],
                    start=(kc == 0),
                    stop=(kc == KC - 1),
                )
            v3 = vones[st][:].rearrange("p (h w) -> p h w", w=65)
            nc.scalar.copy(v3[:, :, 0:64], vps.rearrange("p (h w) -> p h w", w=64))
            nc.scalar.copy(v3[:, :, 64:65], ones6[:].unsqueeze(2))

        def attn_evac(hp, c, yps):
            for h in range(2):
                recip = sb.tile([1, 512], F32, name="recip", tag="recip", bufs=1)
                nc.vector.reciprocal(recip[:], yps[h][64:65, 0:512])
                bc = sb.tile([64, 512], F32, name="bc", tag="bc", bufs=1)
                nc.gpsimd.partition_broadcast(bc[:], recip[:], channels=64)
                nc.vector.tensor_mul(
                    yTn[hp][64 * h : 64 * (h + 1), 512 * c : 512 * (c + 1)],
                    yps[h][0:64, 0:512],
                    bc[:],
                )

        def attn_chunk(hp, c):
            """Causal attention for q-chunk c of head pair hp -> yTn slice."""
            qTt, kTt = qkT[hp], qkT[HP + hp]
            yps = [
                ps.tile([128, 512], F32, name="yps", tag="yT", bufs=2) for _ in range(2)
            ]
            for kb in range(4 * c + 4):
                off = max(0, 128 * kb - 512 * c)
                qsl = slice(512 * c + off, 512 * (c + 1))
                ksl = slice(128 * kb, 128 * (kb + 1))
                sT = ps.tile([128, 1024], F32, name="sT", tag="sT", bufs=2)
                nc.tensor.matmul(
                    sT[:, off:512], kTt[0:64, ksl], qTt[0:64, qsl],
                    start=True, stop=True, tile_position=(0, 0),
                )
                nc.tensor.matmul(
                    sT[:, 512 + off : 1024], kTt[64:128, ksl], qTt[64:128, qsl],
                    start=True, stop=True, tile_position=(64, 0),
                )
                eT = sb.tile([128, 1024], F16, name="eT", tag="eT", bufs=6)
                in3 = sT[:].rearrange("p (b w) -> p b w", b=2)[:, :, off:512]
                out3 = eT[:].rearrange("p (b w) -> p b w", b=2)[:, :, off:512]
                nc.scalar.activation(
                    out3, in3, mybir.ActivationFunctionType.Exp, scale=D**-0.5
                )
                if kb >= 4 * c:  # diagonal block: causal mask multiply
                    for h in range(2):
                        dsl = slice(512 * h + off, 512 * h + off + 128)
                        nc.gpsimd.tensor_mul(eT[:, dsl], eT[:, dsl], mask01[:])
                for h in range(2):
                    nc.tensor.matmul(
                        yps[h][0:65, off:512],
                        vones[kb][:, 65 * (2 * hp + h) : 65 * (2 * hp + h) + 65],
                        eT[:, 512 * h + off : 512 * (h + 1)],
                        start=(kb == 0),
                        stop=(kb == 4 * c + 3),
                    )
            return yps

        # interleaved emission: attention preferred (high_priority) as soon as
        # its deps allow; qk/v projection tiles gap-fill the PE
        def attn_hi(hp, c):
            with tc.high_priority(offset=150):
                yps = attn_chunk(hp, c)
            attn_evac(hp, c, yps)

        qk_sc(0, 0)
        qk_sc(3, 0)
        qk_sc(1, 0)
        qk_sc(4, 0)
        for st in range(0, 4):
            v_tile(st)
        attn_hi(0, 0)
        qk_sc(2, 0)
        qk_sc(5, 0)
        qk_sc(0, 1)
        qk_sc(3, 1)
        qk_sc(1, 1)
        qk_sc(4, 1)
        for st in range(4, 8):
            v_tile(st)
        attn_hi(1, 0)
        attn_hi(0, 1)
        qk_sc(2, 1)
        qk_sc(5, 1)
        qk_sc(0, 2)
        qk_sc(3, 2)
        qk_sc(1, 2)
        qk_sc(4, 2)
        for st in range(8, 12):
            v_tile(st)
        attn_hi(2, 0)
        attn_hi(1, 1)
        attn_hi(0, 2)
        qk_sc(2, 2)
        qk_sc(5, 2)
        qk_sc(0, 3)
        qk_sc(3, 3)
        qk_sc(1, 3)
        qk_sc(4, 3)
        for st in range(12, 16):
            v_tile(st)
        attn_hi(2, 1)
        attn_hi(1, 2)
        attn_hi(0, 3)
        qk_sc(2, 3)
        qk_sc(5, 3)
        attn_hi(2, 2)
        attn_hi(1, 3)
        attn_hi(2, 3)

        wp = [
            sb.tile([128, 768], F32R, name=f"wp{i}", tag="wp", bufs=HP)
            for i in range(HP)
        ]
        for i in range(HP):
            nc.sync.dma_start(wp[i][:], wp_d[128 * i : 128 * (i + 1), :])

        def proj_tile(st):
            osb = sb.tile([128, 768], F32, name="osb", tag="xT", bufs=KC)
            for half in range(2):
                ops_t = ps.tile([128, 512], F32, name="ops", tag="pq", bufs=2)
                ops_ = ops_t[:, 0:384]
                for t in range(HP):
                    nc.tensor.matmul(
                        ops_,
                        yTn[t][:, 128 * st : 128 * (st + 1)],
                        wp[t][:, 384 * half : 384 * (half + 1)],
                        start=(t == 0),
                        stop=(t == HP - 1),
                    )
                nc.vector.tensor_copy(osb[:, 384 * half : 384 * (half + 1)], ops_)
            nc.sync.dma_start(out_d[128 * st : 128 * (st + 1), :], osb[:])

        for st in range(NST):
            proj_tile(st)

    nc.compile()
    return nc


def _rope_tables():
    """cosT/sinA in the even/odd-interleaved d order, tiled to 128 partitions."""
    j = np.arange(32, dtype=np.float64)
    theta = ROPE_BASE ** (-2.0 * j / D)
    pos = np.arange(S, dtype=np.float64)
    freqs = np.outer(theta, pos)  # (32, S)
    cos = np.cos(freqs)
    sin = np.sin(freqs)
    cosT = np.empty((64, S), np.float32)
    sinA = np.empty((64, S), np.float32)
    cosT[0::2] = cos
    cosT[1::2] = cos
    sinA[0::2] = -sin
    sinA[1::2] = sin
    return np.tile(cosT, (2, 1)).copy(), np.tile(sinA, (2, 1)).copy()


def _head_perm():
    """Even/odd interleave of RoPE partner dims, per head (384 cols)."""
    perm = np.empty(384, np.int64)
    for h in range(HPC):
        for j in range(32):
            perm[64 * h + 2 * j] = 64 * h + j
            perm[64 * h + 2 * j + 1] = 64 * h + j + 32
    return perm


def make_in_maps(x, Wqkv, Wproj):
    x = np.asarray(x, np.float32)
    Wqkv = np.asarray(Wqkv, np.float32)
    Wproj = np.asarray(Wproj, np.float32)
    wq, wk, wv = Wqkv[:, 0:C], Wqkv[:, C : 2 * C], Wqkv[:, 2 * C : 3 * C]
    cosT, sinA = _rope_tables()
    perm = _head_perm()
    mask01 = (np.arange(128)[None, :] >= np.arange(128)[:, None]).astype(
        np.float16
    )
    in_maps = []
    for core in range(8):
        b, hh = core // 2, core % 2
        cols = slice(384 * hh, 384 * (hh + 1))
        wq_c = wq[:, cols][:, perm]
        wk_c = wk[:, cols][:, perm]
        in_maps.append(
            {
                "xT": np.ascontiguousarray(x[b].T.astype(np.float16)),
                "wqk": np.ascontiguousarray(np.concatenate([wq_c, wk_c], axis=1).astype(np.float16)),
                "wv": np.ascontiguousarray(wv[:, cols].astype(np.float16)),
                "wp": np.ascontiguousarray(Wproj[384 * hh : 384 * (hh + 1), :]),
                "cosT": cosT.astype(np.float16),
                "sinA": sinA.astype(np.float16),
                "mask01": mask01,
            }
        )
    return in_maps


_NC_CACHE = None


def _get_program():
    global _NC_CACHE
    if _NC_CACHE is None:
        _NC_CACHE = build_program()
    return _NC_CACHE


def kernel(x, Wqkv, Wproj):
    nc = _get_program()
    in_maps = make_in_maps(x, Wqkv, Wproj)
    res = bass_utils.run_bass_kernel_spmd(nc, in_maps, core_ids=list(range(8)))
    out = np.empty((B, S, C), np.float32)
    for b in range(B):
        out[b] = res.results[2 * b]["out"] + res.results[2 * b + 1]["out"]
    return out
